# revision 13
# baseline (speedup 1.0000x reference)
"""Gated spiking reservoir step — Trainium2 Bass kernel (8 NeuronCores).

Math (per reference):
    ic   = inputs @ input_weights                  # [B, R]
    rc   = reservoir_state @ reservoir_weights     # [B, R]
    gate = sigmoid(inputs @ gate_weights)          # [B, R]
    ns   = (0.9 * reservoir_state + 0.1 * tanh(ic + rc)) * gate
    out  = (ns > 0.5) ? 1.0 : 0.0
    returns (out, ns)

Sharding: tensor-parallel over the reservoir (output-column) dim.  Each of
the 8 cores owns a 512-column slice of all three weight matrices and
produces the matching [512-column x full-batch] slice of both outputs.
The activations (inputs / reservoir_state) are replicated, pre-transposed
on host to [K, B] so the contraction dim lands on SBUF partitions.

On-device layout per core (everything transposed — state tiles are
[cols(part) x batch(free)]):
    for each batch slice of 512:
        gate_ps[c]  += w_gate[k-tile, c-tile].T @ xT[k-tile, b-slice]   (8 k-tiles)
        state_ps[c] += w_in  [k-tile, c-tile].T @ xT[k-tile, b-slice]   (8 k-tiles)
        state_ps[c] += w_res [k-tile, c-tile].T @ sT[k-tile, b-slice]   (32 k-tiles)
        t  = tanh(state_ps[c])        (ScalarE, from PSUM)
        g  = sigmoid(gate_ps[c])      (ScalarE, from PSUM)
        v  = (s_slice * 9.0) + t      (VectorE scalar_tensor_tensor)
        ns = (v * 0.1) * g            (VectorE scalar_tensor_tensor)
        spk = ns > 0.5                (VectorE tensor_scalar is_gt -> 1.0/0.0)
"""

import os
import sys

if "/opt/trn_rl_repo" not in sys.path:
    sys.path.insert(0, "/opt/trn_rl_repo")

import numpy as np

B, D_IN, R = 2048, 1024, 4096
N_CORES = 8
COLS = R // N_CORES          # 512 output columns per core
P = 128                      # SBUF/PSUM partitions
NB = 512                     # batch free-dim per matmul / PSUM bank
KI = D_IN // P               # 8 k-tiles over the input dim
KR = R // P                  # 32 k-tiles over the reservoir dim
CT = COLS // P               # 4 column tiles per core
BT = B // NB                 # 4 batch slices

# float32r runs the PE at 4x the fp32 rate for moving dim >= 256.
MM_DTYPE = os.environ.get("BASS_MM_DTYPE", "float32r")

_CACHE = {}


def _build(mm_dtype_name: str):
    from contextlib import ExitStack

    from concourse import bacc, tile
    import concourse.mybir as mybir

    f32 = mybir.dt.float32
    mm_dt = getattr(mybir.dt, mm_dtype_name)
    AF = mybir.ActivationFunctionType
    ALU = mybir.AluOpType

    nc = bacc.Bacc(
        "TRN2", target_bir_lowering=False, debug=False, enable_asserts=False
    )

    xT = nc.dram_tensor("xT", [D_IN, B], f32, kind="ExternalInput")
    sT = nc.dram_tensor("sT", [R, B], f32, kind="ExternalInput")
    w_in = nc.dram_tensor("w_in", [D_IN, COLS], f32, kind="ExternalInput")
    w_res = nc.dram_tensor("w_res", [R, COLS], f32, kind="ExternalInput")
    w_gate = nc.dram_tensor("w_gate", [D_IN, COLS], f32, kind="ExternalInput")
    nsT = nc.dram_tensor("nsT", [COLS, B], f32, kind="ExternalOutput")
    spkT = nc.dram_tensor("spkT", [COLS, B], mybir.dt.uint8, kind="ExternalOutput")

    def cast(ap):
        return ap.bitcast(mm_dt) if mm_dtype_name != "float32" else ap

    with tile.TileContext(nc) as tc, ExitStack() as ctx:
        # Resident weights: 12 MB of SBUF (96 KB/partition), one tile per
        # 128-row k-slice so matmuls only wait on the slice they consume.
        wpool = ctx.enter_context(tc.tile_pool(name="weights", bufs=1))
        w_in_sb, w_gate_sb, w_res_sb = [], [], []
        for k in range(KI):
            t = wpool.tile([P, COLS], mm_dt, tag=f"w_in_{k}", name=f"w_in_sb{k}")
            nc.gpsimd.dma_start(t[:], cast(w_in[k * P : (k + 1) * P, :]))
            w_in_sb.append(t)
            t = wpool.tile([P, COLS], mm_dt, tag=f"w_gate_{k}", name=f"w_gate_sb{k}")
            nc.gpsimd.dma_start(t[:], cast(w_gate[k * P : (k + 1) * P, :]))
            w_gate_sb.append(t)
        for k in range(KR):
            t = wpool.tile([P, COLS], mm_dt, tag=f"w_res_{k}", name=f"w_res_sb{k}")
            w_res_sb.append(t)

        for k in range(KR):
            nc.gpsimd.dma_start(w_res_sb[k][:], cast(w_res[k * P : (k + 1) * P, :]))

        xpool = ctx.enter_context(tc.tile_pool(name="x_mov", bufs=6))
        spool = ctx.enter_context(tc.tile_pool(name="s_mov", bufs=6))
        st_psum = ctx.enter_context(tc.tile_pool(name="st_ps", bufs=4, space="PSUM"))
        gt_psum = ctx.enter_context(tc.tile_pool(name="gt_ps", bufs=4, space="PSUM"))
        epool = ctx.enter_context(tc.tile_pool(name="epilogue", bufs=3))

        for b in range(BT):
            bs = slice(b * NB, (b + 1) * NB)
            state_ps = [st_psum.tile([P, NB], f32, tag="state", name=f"state_ps_{b}_{i}") for i in range(CT)]
            gate_ps = [gt_psum.tile([P, NB], f32, tag="gate", name=f"gate_ps_{b}_{i}") for i in range(CT)]

            # Gate matmuls first so gate PSUM banks retire early.
            for k in range(KI):
                xt = xpool.tile([P, NB], mm_dt, tag="xt")
                nc.sync.dma_start(xt[:], cast(xT[k * P : (k + 1) * P, bs]))
                for c in range(CT):
                    nc.tensor.matmul(
                        gate_ps[c][:],
                        w_gate_sb[k][:, c * P : (c + 1) * P],
                        xt[:],
                        start=(k == 0),
                        stop=(k == KI - 1),
                    )
                for c in range(CT):
                    nc.tensor.matmul(
                        state_ps[c][:],
                        w_in_sb[k][:, c * P : (c + 1) * P],
                        xt[:],
                        start=(k == 0),
                        stop=False,
                    )
            # s-phase A: first half of the k-tiles, all column tiles in
            # lockstep (k-major) so each st tile is short-lived.
            KH = KR // 2
            st_ep = []
            for k in range(KH):
                if k < CT:
                    # This core's own state rows (epilogue reads them too):
                    # keep an exact fp32 copy, round to f32r on-chip for PE.
                    sf = spool.tile([P, NB], f32, tag="stEp", bufs=9,
                                    name=f"stEp_{b}_{k}")
                    nc.sync.dma_start(sf[:], sT[k * P : (k + 1) * P, bs])
                    st_ep.append(sf)
                    if mm_dtype_name != "float32":
                        st = spool.tile([P, NB], mm_dt, tag="stEpR", bufs=3,
                                        name=f"stEpR_{b}_{k}")
                        nc.sync.dma_start(st[:], sf[:].bitcast(mm_dt))
                    else:
                        st = sf
                else:
                    st = spool.tile([P, NB], mm_dt, tag="st")
                    nc.sync.dma_start(st[:], cast(sT[k * P : (k + 1) * P, bs]))
                for c in range(CT):
                    nc.tensor.matmul(
                        state_ps[c][:],
                        w_res_sb[k][:, c * P : (c + 1) * P],
                        st[:],
                        start=False,
                        stop=False,
                    )
            # s-phase B: second half column-major, so state_ps[c] finishes
            # (and its PSUM slot frees via tanh) staggered well before the
            # slice ends -- removes the PE bubble at slice boundaries.
            stB = []
            for k in range(KH, KR):
                st = spool.tile([P, NB], mm_dt, tag="stB", bufs=18, name=f"stB_{b}_{k}")
                nc.sync.dma_start(st[:], cast(sT[k * P : (k + 1) * P, bs]))
                stB.append(st)
            for c in range(CT):
                for j, k in enumerate(range(KH, KR)):
                    nc.tensor.matmul(
                        state_ps[c][:],
                        w_res_sb[k][:, c * P : (c + 1) * P],
                        stB[j][:],
                        start=False,
                        stop=(k == KR - 1),
                    )

            NH = NB // 2
            for c in range(CT):
                cs = slice(c * P, (c + 1) * P)
                se_f32 = st_ep[c]
                ns = epool.tile([P, NB], f32, tag="ns", name=f"ns_{b}_{c}")
                spk = epool.tile([P, NB], mybir.dt.uint8, tag="spk",
                                 name=f"spk_{b}_{c}")
                for h in range(2):
                    hs = slice(h * NH, (h + 1) * NH)
                    tt = epool.tile([P, NH], f32, tag="tanh")
                    nc.scalar.activation(tt[:], state_ps[c][:, hs], AF.Tanh)
                    gg = epool.tile([P, NH], f32, tag="sig")
                    nc.scalar.activation(gg[:], gate_ps[c][:, hs], AF.Sigmoid)
                    vv = epool.tile([P, NH], f32, tag="v")
                    nc.vector.scalar_tensor_tensor(
                        vv[:], se_f32[:, hs], 9.0, tt[:], ALU.mult, ALU.add
                    )
                    nc.vector.scalar_tensor_tensor(
                        ns[:, hs], vv[:], 0.1, gg[:], ALU.mult, ALU.mult
                    )
                    nc.vector.tensor_scalar(
                        spk[:, hs], ns[:, hs], 0.5, None, ALU.is_gt
                    )
                nc.sync.dma_start(nsT[cs, bs], ns[:])
                nc.sync.dma_start(spkT[cs, bs], spk[:])

    nc.compile()
    return nc


def _get_program():
    if MM_DTYPE not in _CACHE:
        _CACHE[MM_DTYPE] = _build(MM_DTYPE)
    return _CACHE[MM_DTYPE]


def kernel(inputs, prev_output, reservoir_state, input_weights, reservoir_weights,
           gate_weights):
    from concourse.bass_utils import run_bass_kernel_spmd

    nc = _get_program()

    x = np.ascontiguousarray(np.asarray(inputs, dtype=np.float32))
    s = np.ascontiguousarray(np.asarray(reservoir_state, dtype=np.float32))
    w_in = np.ascontiguousarray(np.asarray(input_weights, dtype=np.float32))
    w_res = np.ascontiguousarray(np.asarray(reservoir_weights, dtype=np.float32))
    w_gate = np.ascontiguousarray(np.asarray(gate_weights, dtype=np.float32))

    xT = np.ascontiguousarray(x.T)          # [D_IN, B]
    sT = np.ascontiguousarray(s.T)          # [R, B]

    in_maps = []
    for core in range(N_CORES):
        c0 = core * COLS
        cs = slice(c0, c0 + COLS)
        # Rotate the contraction (reservoir-row) order so this core's own
        # 512 state rows arrive as k-tiles 0..3 -- the epilogue reuses those
        # SBUF tiles directly instead of re-reading them from HBM.  The same
        # rotation is applied to w_res rows, so the dot products are
        # unchanged (summation is commutative).
        w_res_c = w_res[:, cs]
        in_maps.append(
            {
                "xT": xT,
                "sT": np.concatenate([sT[c0:], sT[:c0]], axis=0),
                "w_in": np.ascontiguousarray(w_in[:, cs]),
                "w_res": np.concatenate([w_res_c[c0:], w_res_c[:c0]], axis=0),
                "w_gate": np.ascontiguousarray(w_gate[:, cs]),
            }
        )

    res = run_bass_kernel_spmd(nc, in_maps, list(range(N_CORES)))

    ns_T = np.concatenate([res.results[c]["nsT"] for c in range(N_CORES)], axis=0)
    spk_T = np.concatenate([res.results[c]["spkT"] for c in range(N_CORES)], axis=0)
    new_state = np.ascontiguousarray(ns_T.T)     # [B, R]
    output = spk_T.T.astype(np.float32)          # [B, R]
    return output, new_state
